# revision 6
# baseline (speedup 1.0000x reference)
"""Distributed Trainium2 kernel for nn_Attention_72722386256499 — v3.

v3 changes vs v2 baseline (181.7us):
- QK^T uses PE row-tiling (64x128 mode, tiles T0/T8): the two heads of a
  group run CONCURRENTLY on the two row-halves of the PE array.
  Measured: 280ns/pair vs 852ns untiled.
- Main loop is phase-grouped: 8-unit QK phases (tiled mode) software-
  pipelined against 8-unit PV phases (128x128 mode) to amortize the
  ~85ns mode-switch drain.
- exp is split: ACT does sim cols [0,EXPC), DVE does [EXPC,1024) via a
  Schraudolph int16 trick (i16 = 1477.32*x + 15315 bit-cast to fp16,
  rel err +-3%) writing straight into the fp16 ex tile.
- Everything downstream of exp is fp16 (ex, vT1, o2T, out2, outrs, wo,
  fouts, out_ext): fp16 matmuls run at bf16 speed, and fp16's 2^-11
  mantissa recovers precision spent on the Schraudolph share.
"""

import sys

sys.path.insert(0, "/opt/trn_rl_repo")

import math

import ml_dtypes
import numpy as np

import bass_rust
import concourse.bass as bass
import concourse.mybir as mybir
import concourse.tile as tile
from concourse import masks
from concourse.bass_utils import run_bass_kernel_spmd

B, C, L = 4, 512, 2048
H, D = 8, 64
HPC = 4  # heads per core
NCORES = 8
FP = mybir.dt.float32
F16 = mybir.dt.float16
I16 = mybir.dt.int16
BF = mybir.dt.bfloat16

NJ = L // 128  # 16 j tiles
NIB = L // 512  # 4 i blocks
NCC = C // 128  # 4 contraction chunks

# exp column split: ACT handles [0, EXPC), DVE-Schraudolph [EXPC, 1024)
EXPC = 512
A16_S = 1024.0 / math.log(2.0)
B16_S = 15.0 * 1024.0 - 45.0

TRACE_MODE = False
LAST_RESULT = None
_NC_CACHE = {}


def _split_waits(nc, max_waits=1):
    """walrus rejects >1 sync wait per instruction; hoist extras onto
    single-wait NoOps just before the instruction on the same engine."""
    counter = 0
    for f in nc.m.functions:
        for bb in f.blocks:
            il = bb.instructions
            new_list = []
            changed = False
            for inst in il:
                si = inst.sync_info
                if si is None:
                    new_list.append(inst)
                    continue
                waits = list(si.on_wait)
                if len(waits) > max_waits:
                    keep = waits[-max_waits:]
                    for w in waits[:-max_waits]:
                        counter += 1
                        nop = mybir.InstNoOp(
                            name=f"I-waitsplit-{counter}", ins=[], outs=[]
                        )
                        nop.engine = inst.engine
                        nop.sync_info = bass_rust.SyncInfo(on_wait=[w], on_update=[])
                        new_list.append(nop)
                        nc.register_instruction(nop, overwrite=True)
                    inst.sync_info = bass_rust.SyncInfo(
                        on_wait=keep, on_update=list(si.on_update)
                    )
                    changed = True
                new_list.append(inst)
            if changed:
                il.clear()
                il.extend(new_list)
    return counter


def build_nc(debug=False):
    nc = bass.Bass()
    # Host-prepacked layouts: every DRAM line is contiguous per partition.
    x_ext = nc.declare_dram_parameter("x", [128, NIB, NCC, 512], BF, isOutput=False)
    wq_ext = nc.declare_dram_parameter("wq", [128, NCC, HPC * D], BF, isOutput=False)
    wk_ext = nc.declare_dram_parameter("wk", [128, NCC, HPC * D], BF, isOutput=False)
    wv_ext = nc.declare_dram_parameter("wv", [128, NCC, HPC * D], BF, isOutput=False)
    wo_ext = nc.declare_dram_parameter("wo", [128, 2, C], F16, isOutput=False)
    out_ext = nc.declare_dram_parameter("out", [C, L], F16, isOutput=True)
    if debug:
        dbg_q = nc.declare_dram_parameter("dbg_q", [128, 2, L], BF, isOutput=True)
        dbg_k = nc.declare_dram_parameter("dbg_k", [128, 2, L], BF, isOutput=True)
        dbg_v = nc.declare_dram_parameter(
            "dbg_v", [128, 4, 4, HPC, D + 1], F16, isOutput=True
        )
        dbg_o2 = nc.declare_dram_parameter(
            "dbg_o2", [128, HPC, NJ, D], F16, isOutput=True
        )
        dbg_rs = nc.declare_dram_parameter("dbg_rs", [128, 2, L], F16, isOutput=True)

    with tile.TileContext(nc) as tc:
        with (
            tc.tile_pool(name="const", bufs=1) as cpool,
            tc.tile_pool(name="exp", bufs=18) as epool,
            tc.tile_pool(name="o2", bufs=8) as o2pool,
            tc.tile_pool(name="rz", bufs=8) as rzpool,
            tc.tile_pool(name="fout", bufs=4) as fpool,
            tc.tile_pool(name="ps", bufs=3, space="PSUM") as ppool,
            tc.tile_pool(name="po", bufs=2, space="PSUM") as opool,
        ):
            # ---- persistent SBUF tensors ----
            x_sb = cpool.tile([128, NIB, NCC, 512], BF, tag="x")
            wq_sb = cpool.tile([128, NCC, HPC * D], BF, tag="wq")
            wk_sb = cpool.tile([128, NCC, HPC * D], BF, tag="wk")
            wv_sb = cpool.tile([128, NCC, HPC * D], BF, tag="wv")
            wo_sb = cpool.tile([128, 2, C], F16, tag="wo")
            q_sbs = [
                cpool.tile([128, L], BF, tag=f"q{g}", name=f"q_sb{g}")
                for g in range(2)
            ]
            k_sbs = [
                cpool.tile([128, L], BF, tag=f"k{g}", name=f"k_sb{g}")
                for g in range(2)
            ]
            vT1s = [
                cpool.tile([128, 4, HPC, D + 1], F16, tag=f"vT1{qt}", name=f"vT1_sb{qt}")
                for qt in range(4)
            ]
            out2_sb = cpool.tile([128, HPC, NJ, D], F16, tag="out2")
            outrs_sbs = [
                cpool.tile([128, L], F16, tag=f"outrs{g}", name=f"outrs_sb{g}")
                for g in range(2)
            ]
            ident = cpool.tile([128, 128], F16, tag="ident")

            # ---- input DMAs, critical-first, spread over engine queues ----
            nc.scalar.dma_start(out=wq_sb, in_=wq_ext.ap())
            nc.sync.dma_start(out=wk_sb, in_=wk_ext.ap())
            nc.gpsimd.dma_start(out=x_sb[:, 0, 0:2], in_=x_ext[:, 0, 0:2])
            nc.sync.dma_start(out=x_sb[:, 0, 2:4], in_=x_ext[:, 0, 2:4])
            nc.scalar.dma_start(out=wv_sb, in_=wv_ext.ap())
            nc.gpsimd.dma_start(out=x_sb[:, 1, 0:2], in_=x_ext[:, 1, 0:2])
            nc.scalar.dma_start(out=x_sb[:, 1, 2:4], in_=x_ext[:, 1, 2:4])
            nc.gpsimd.dma_start(out=x_sb[:, 2, 0:2], in_=x_ext[:, 2, 0:2])
            nc.sync.dma_start(out=x_sb[:, 2, 2:4], in_=x_ext[:, 2, 2:4])
            nc.scalar.dma_start(out=x_sb[:, 3, 0:2], in_=x_ext[:, 3, 0:2])
            nc.sync.dma_start(out=x_sb[:, 3, 2:4], in_=x_ext[:, 3, 2:4])
            nc.scalar.dma_start(out=wo_sb, in_=wo_ext.ap())

            masks.make_identity(nc, ident[:, :])
            ones_f16 = cpool.tile([128, 4 * HPC], F16, tag="ones")
            nc.vector.memset(ones_f16, 1.0)
            exp_warm = cpool.tile([128, 16], F16, tag="expwarm")
            nc.scalar.activation(
                out=exp_warm,
                in_=ones_f16[:, 0:16],
                func=mybir.ActivationFunctionType.Exp,
            )
            for qt in range(4):
                nc.vector.tensor_copy(
                    out=vT1s[qt][:, :, :, D : D + 1],
                    in_=ones_f16.rearrange("p (a b) -> p a b", b=HPC).unsqueeze(-1),
                )

            # ---- projection task emitters (128x128 mode) ----
            sab_rr = [0]

            def s_tag():
                sab_rr[0] += 1
                return "sA" if sab_rr[0] % 2 else "sB"

            def qk_group(w_sb, g, dst, lb, eng="vector"):
                def t():
                    ps = ppool.tile([128, 512], FP, tag=s_tag())
                    for ci in range(NCC):
                        nc.tensor.matmul(
                            ps,
                            w_sb[:, ci, g * 128 : (g + 1) * 128],
                            x_sb[:, lb, ci, :],
                            start=(ci == 0),
                            stop=(ci == NCC - 1),
                        )
                    dsl = dst[:, lb * 512 : (lb + 1) * 512]
                    if eng == "scalar":
                        nc.scalar.copy(out=dsl, in_=ps)
                    else:
                        nc.vector.tensor_copy(out=dsl, in_=ps)
                return t

            def v_sub(j):
                # vT1 for j-tile j (all 4 heads)
                def t():
                    qt, j4 = j // 4, j % 4
                    pst = ppool.tile([128, 512], FP, tag=s_tag())
                    ps = pst[:, 0 : HPC * D]
                    for ci in range(NCC):
                        nc.tensor.matmul(
                            ps,
                            x_sb[:, qt, ci, j4 * 128 : (j4 + 1) * 128],
                            wv_sb[:, ci, :],
                            start=(ci == 0),
                            stop=(ci == NCC - 1),
                        )
                    nc.vector.tensor_copy(
                        out=vT1s[qt][:, j4, :, 0:D],
                        in_=ps.rearrange("p (h d) -> p h d", h=HPC),
                    )
                return t

            # HAM warmup: PE busy on garbage while input DMAs land
            warm_ps = ppool.tile([128, 512], FP, tag="sA")
            for _ in range(4):
                nc.tensor.matmul(
                    warm_ps[0:64, :],
                    out2_sb[:, 0, 0, 0:64],
                    out2_sb.rearrange("p a b c -> p (a b c)")[:, 0:512],
                    start=True,
                    stop=True,
                )
            for _ in range(40):
                nc.tensor.ldweights(out2_sb[:, 0, 0, 0:64])

            # minimal prework for QK-group 0 (block 0, jt 0-7):
            # q g0 lb0, k g0 lb0+lb1
            qk_group(wq_sb, 0, q_sbs[0], 0, "scalar")()
            qk_group(wk_sb, 0, k_sbs[0], 0, "vector")()
            qk_group(wk_sb, 0, k_sbs[0], 1, "scalar")()

            # deferred tasks keyed by QK unit u: run just BEFORE
            # emit_qk_tiled(u).  v_sub(j) must precede PV unit j (which runs
            # at u >= j+LAG); qk/k chains must precede the QK units that read
            # their q/k columns.
            qk_tasks = {}

            def add_qk_task(u, t):
                qk_tasks.setdefault(u, []).append(t)

            for j in range(16):
                add_qk_task(j + 2, v_sub(j))
            add_qk_task(6, qk_group(wk_sb, 0, k_sbs[0], 2, "vector"))
            add_qk_task(7, qk_group(wk_sb, 0, k_sbs[0], 3, "scalar"))
            add_qk_task(12, qk_group(wq_sb, 0, q_sbs[0], 1))
            add_qk_task(28, qk_group(wq_sb, 0, q_sbs[0], 2))
            add_qk_task(44, qk_group(wq_sb, 0, q_sbs[0], 3))
            add_qk_task(57, qk_group(wk_sb, 1, k_sbs[1], 0))
            add_qk_task(59, qk_group(wq_sb, 1, q_sbs[1], 0, "scalar"))
            add_qk_task(61, qk_group(wk_sb, 1, k_sbs[1], 1))
            add_qk_task(66, qk_group(wk_sb, 1, k_sbs[1], 2, "scalar"))
            add_qk_task(68, qk_group(wk_sb, 1, k_sbs[1], 3))
            add_qk_task(76, qk_group(wq_sb, 1, q_sbs[1], 1))
            add_qk_task(92, qk_group(wq_sb, 1, q_sbs[1], 2))
            add_qk_task(108, qk_group(wq_sb, 1, q_sbs[1], 3))

            # ---- flush machinery ----
            pending = []
            dma_rr = [0]

            def make_flush_unit(h, ib2, cc, o2T):
                def u():
                    tg = ib2 * 4 + cc
                    ps_t = ppool.tile([128, 512], F16, tag=s_tag())
                    nc.tensor.transpose(
                        ps_t[:, 0:128],
                        o2T[:, cc * 128 : (cc + 1) * 128],
                        ident[:, :],
                    )
                    rz = rzpool.tile([128, 1], FP, tag="rz")
                    nc.vector.reciprocal(out=rz, in_=ps_t[:, D : D + 1])
                    if (tg + h) % 2:
                        nc.scalar.activation(
                            out=out2_sb[:, h, tg, :],
                            in_=ps_t[:, 0:D],
                            func=mybir.ActivationFunctionType.Copy,
                            scale=rz,
                        )
                    else:
                        nc.vector.tensor_scalar_mul(
                            out=out2_sb[:, h, tg, :],
                            in0=ps_t[:, 0:D],
                            scalar1=rz,
                        )
                    # SBUF->SBUF scramble: out2[(pa pb), d] ->
                    # outrs rows (h%2)*64 + 4*tg + pa, cols pb*64 + d
                    r0 = (h % 2) * 64 + 4 * tg
                    engs = (nc.sync, nc.gpsimd)
                    weng = engs[dma_rr[0] % len(engs)]
                    dma_rr[0] += 1
                    weng.dma_start(
                        out=outrs_sbs[h // 2][r0 : r0 + 4, :].rearrange(
                            "q (b d) -> q b d", d=D
                        ),
                        in_=out2_sb[:, h, tg, :],
                    )
                return u

            def emit_qk_tiled(u):
                """Row-tiled QK pair for unit u: T0 (head hp0) -> sA bank,
                T8 (hp1) -> sB bank; ACT exps sA, DVE-Schraudolph eats sB."""
                g, ib, jt = u // 64, (u // 16) % 4, u % 16
                i0 = ib * 512
                ps_a = ppool.tile([128, 512], FP, tag="sA")
                ps_b = ppool.tile([128, 512], FP, tag="sB")
                for hp, ps in ((0, ps_a), (1, ps_b)):
                    p0 = hp * 64
                    nc.tensor.matmul(
                        ps,
                        k_sbs[g][p0 : p0 + 64, jt * 128 : (jt + 1) * 128],
                        q_sbs[g][p0 : p0 + 64, i0 : i0 + 512],
                        start=True,
                        stop=True,
                        tile_position=(p0, 0),
                    )
                ex = epool.tile([128, 1024], F16, tag="exp")
                nc.scalar.activation(
                    out=ex[:, 0:512],
                    in_=ps_a,
                    func=mybir.ActivationFunctionType.Exp,
                )
                nc.vector.tensor_scalar(
                    out=ex[:, 512:1024].bitcast(I16),
                    in0=ps_b,
                    scalar1=A16_S,
                    scalar2=B16_S,
                    op0=mybir.AluOpType.mult,
                    op1=mybir.AluOpType.add,
                )
                return ex

            # ---- main loop: 16 supers of [QK-group(s); PV-group(s-1)] ----
            exs = {}
            o_tiles = None
            o2_eng = [0]

            def emit_pv(u):
                nonlocal o_tiles
                g, ib, jt = u // 64, (u // 16) % 4, u % 16
                if jt == 0:
                    ps_oA = opool.tile([128, 512], FP, tag="o")
                    ps_oB = opool.tile([128, 512], FP, tag="o")
                    o_tiles = (ps_oA, ps_oB)
                ex = exs.pop(u)
                for hp in range(2):
                    nc.tensor.matmul(
                        o_tiles[hp][0 : D + 1, :],
                        vT1s[jt // 4][:, jt % 4, 2 * g + hp, :],
                        ex[:, hp * 512 : (hp + 1) * 512],
                        start=(jt == 0),
                        stop=(jt == NJ - 1),
                    )
                if jt == NJ - 1:
                    for hp in range(2):
                        h = 2 * g + hp
                        o2T = o2pool.tile([128, 512], F16, tag="o2T")
                        if o2_eng[0] % 2 == 0:
                            nc.vector.tensor_copy(
                                out=o2T[0 : D + 1, :], in_=o_tiles[hp][0 : D + 1, :]
                            )
                        else:
                            nc.scalar.copy(
                                out=o2T[0 : D + 1, :], in_=o_tiles[hp][0 : D + 1, :]
                            )
                        o2_eng[0] += 1
                        for cc in range(4):
                            pending.append((h, ib, cc, o2T))

            LAG = 6
            pv_next = [0]

            def pv_batch(limit):
                if pv_next[0] < limit and pending:
                    make_flush_unit(*pending.pop(0))()
                while pv_next[0] < limit:
                    v = pv_next[0]
                    if v % 4 == 0 and pending:
                        make_flush_unit(*pending.pop(0))()
                    emit_pv(v)
                    pv_next[0] += 1

            for u in range(128):
                for t in qk_tasks.get(u, ()):
                    t()
                exs[u] = emit_qk_tiled(u)
                if u % 2 == 1:
                    pv_batch(max(0, u - LAG + 1))
            pv_batch(128)

            fouts = [
                fpool.tile([128, L], F16, tag="fout", name=f"fo{og}")
                for og in range(4)
            ]
            engs3 = (nc.sync, nc.gpsimd, nc.scalar)
            dq = [0]
            ogtiles = {}

            def rc0_chain(og, tags):
                ogtiles[og] = []
                for lb in range(4):
                    if tags[lb] == "o":
                        ps = opool.tile([128, 512], FP, tag="o",
                                        name=f"rcp_{og}_{lb}")
                    else:
                        ps = ppool.tile([128, 512], FP, tag=tags[lb],
                                        name=f"rcp_{og}_{lb}")
                    ogtiles[og].append(ps)
                    nc.tensor.matmul(
                        ps,
                        wo_sb[:, 0, og * 128 : (og + 1) * 128],
                        outrs_sbs[0][:, lb * 512 : (lb + 1) * 512],
                        start=True,
                        stop=False,
                    )

            def finish(og):
                for lb in range(4):
                    nc.tensor.matmul(
                        ogtiles[og][lb],
                        wo_sb[:, 1, og * 128 : (og + 1) * 128],
                        outrs_sbs[1][:, lb * 512 : (lb + 1) * 512],
                        start=False,
                        stop=True,
                    )
                    it = og * 4 + lb
                    fsl = fouts[og][:, lb * 512 : (lb + 1) * 512]
                    if it % 2 == 0:
                        nc.scalar.copy(out=fsl, in_=ogtiles[og][lb])
                    else:
                        nc.vector.tensor_copy(out=fsl, in_=ogtiles[og][lb])
                    engs3[dq[0] % 3].dma_start(
                        out=out_ext[og * 128 : og * 128 + 128,
                                    lb * 512 : (lb + 1) * 512],
                        in_=fsl,
                    )
                    dq[0] += 1

            # og0 rc0 chains open before the drain (banks: o,o,sA,sB),
            # leaving one sA + one sB buffer rotating for drain transposes
            rc0_chain(0, ("o", "o", "sA", "sB"))
            while pending:
                make_flush_unit(*pending.pop(0))()
            rc0_chain(1, ("sA", "sB", "sA", "sB"))
            finish(0)
            rc0_chain(2, ("o", "o", "sA", "sB"))
            finish(1)
            rc0_chain(3, ("sA", "sB", "sA", "sB"))
            finish(2)
            finish(3)

            if debug:
                for g in range(2):
                    nc.sync.dma_start(out=dbg_q.ap()[:, g], in_=q_sbs[g])
                    nc.sync.dma_start(out=dbg_k.ap()[:, g], in_=k_sbs[g])
                    nc.sync.dma_start(out=dbg_rs.ap()[:, g], in_=outrs_sbs[g])
                for qt in range(4):
                    nc.sync.dma_start(out=dbg_v.ap()[:, qt], in_=vT1s[qt])
                nc.sync.dma_start(out=dbg_o2.ap(), in_=out2_sb)

    _split_waits(nc)
    return nc


def _get_nc():
    if "v3" not in _NC_CACHE:
        _NC_CACHE["v3"] = build_nc()
    return _NC_CACHE["v3"]


def _prepack_x(xb):
    # x[b] (C, L) -> [p, lb, ci, n] with c = ci*128+p, l = lb*512+n
    return np.ascontiguousarray(
        xb.reshape(NCC, 128, NIB, 512).transpose(1, 2, 0, 3)
    )


def _prepack_w(w):
    # (C, 256) -> [p, ci, n]
    return np.ascontiguousarray(w.reshape(NCC, 128, HPC * D).transpose(1, 0, 2))


def _prepack_wo(w):
    # (256, C) -> [p, rc, o]
    return np.ascontiguousarray(w.reshape(2, 128, C).transpose(1, 0, 2))


def kernel(x, w_qkv, w_out, b_out):
    global LAST_RESULT
    x = np.asarray(x, dtype=np.float32)
    w_qkv = np.asarray(w_qkv, dtype=np.float32)
    w_out = np.asarray(w_out, dtype=np.float32)
    b_out = np.asarray(b_out, dtype=np.float32)

    scale = D**-0.5
    bf16 = ml_dtypes.bfloat16
    xs = [_prepack_x(x[b]).astype(bf16) for b in range(B)]
    in_maps = []
    for m in range(NCORES):
        b = m // 2
        hs = [4 * (m % 2) + i for i in range(HPC)]
        q_rows = np.concatenate([np.arange(h * D, (h + 1) * D) for h in hs])
        wq = np.ascontiguousarray((w_qkv[q_rows, :] * scale).T)
        wk = np.ascontiguousarray(w_qkv[C + q_rows, :].T)
        wv = np.ascontiguousarray(w_qkv[2 * C + q_rows, :].T)
        wo = np.ascontiguousarray(w_out[:, q_rows].T)
        in_maps.append(
            {
                "x": xs[b],
                "wq": _prepack_w(wq).astype(bf16),
                "wk": _prepack_w(wk).astype(bf16),
                "wv": _prepack_w(wv).astype(bf16),
                "wo": _prepack_wo(wo).astype(np.float16),
            }
        )

    nc = _get_nc()
    res = run_bass_kernel_spmd(
        nc, in_maps, core_ids=list(range(NCORES)), trace=TRACE_MODE
    )
    LAST_RESULT = res

    out = np.empty((B, C, L), dtype=np.float32)
    for b in range(B):
        out[b] = res.results[2 * b]["out"].astype(np.float32) + res.results[
            2 * b + 1
        ]["out"].astype(np.float32)
        out[b] += b_out[:, None]
    return out
